# revision 4
# baseline (speedup 1.0000x reference)
"""Trainium2 Bass kernel for nn_DP_CAML_33646773797448 (sparse_attention).

Reference computation (per batch b):
    e      = embed_w[ids[b]]                       # (T, D)
    x      = e.T                                   # (D, T)
    h      = relu(conv1d(x, conv_w, pad=K-1) + b)  # (D, T')  T' = T + K - 1
    s      = U @ h                                 # (L, T')  raw scores
    attn   = softmax(s, axis=-1)
    z      = attn @ h.T                            # (L, D)
    logits = sum_d z * U + fc_bias                 # (L,)

Key identity: logits[l] = sum_t softmax(s)[l,t] * s[l,t], i.e. a
softmax-weighted mean of the raw scores. The kernel computes, per l-tile of
128 rows, den[l] = sum_t exp(s) (scalar-engine accumulate during the exp
pass) and num[l] = sum_t s*exp(s) (vector-engine scalar_tensor_tensor with
accumulate). The last 9 score columns (t=2048..2056) are computed by a
separate transposed matmul (stationary = h_tail so the weight loads are
9-column, i.e. free) and the raw 9 x L scores are shipped to the host, which
does the final (num+num_tail)/(den+den_tail) + bias combine while stacking
the 8 per-core outputs.

Sharding: pure data-parallel over B (B == 8 == n_cores), no collectives.
"""

import numpy as np

import concourse.bass as bass
import concourse.tile as tile
from concourse import bacc
from concourse import mybir
from concourse.bass_utils import run_bass_kernel_spmd
from concourse.masks import make_identity

F32 = mybir.dt.float32
F16 = mybir.dt.float16
BF16 = mybir.dt.bfloat16
I32 = mybir.dt.int32

# Problem shapes (hardcoded per contract)
VOCAB, L, D, K = 50000, 8921, 300, 10
B, T = 8, 2048
TP = T + K - 1            # 2057 conv output length
TP_PAD = TP + 1           # 2058 (the junk col 2057 is never read by stats)
NTB = 17                  # gather blocks of 128 tokens (ids padded with token 0)
T_G = NTB * 128           # 2176 gathered tokens
DPAR = [128, 128, 44]     # D = 300 split into partition chunks
NDC = 3
LT = (L + 127) // 128     # 70 l-tiles (last one has 89 valid rows)
L_PAD = LT * 128          # 8960
D_PAD = NDC * 128         # 384
TW = 9                    # tail columns t = 2048..2056

# conv t'-blocks (equal-ish, <= 512 so each fits a PSUM bank)
CONV_BLOCKS = [(0, 412), (412, 412), (824, 412), (1236, 412), (1648, 410)]
# gathers that must be complete before conv block i can run:
# block i needs x cols [t0, t0+tw+K-1) -> tokens up to t0+tw+8
CONV_NEED_TB = [4, 7, 10, 13, 17]

_BUILT = {}


def _build_bass():
    nc = bacc.Bacc("TRN2", target_bir_lowering=False, debug=False)

    ids_d = nc.dram_tensor("ids", [T_G], I32, kind="ExternalInput").ap()
    emb_d = nc.dram_tensor("embed_w", [VOCAB, D], F32, kind="ExternalInput").ap()
    w_d = nc.dram_tensor("w_prep", [NDC, 128, K * D_PAD], F16, kind="ExternalInput").ap()
    cb_d = nc.dram_tensor("cb_prep", [NDC, 128], F32, kind="ExternalInput").ap()
    ut_d = nc.dram_tensor("ut_prep", [128, LT * NDC * 128], F16, kind="ExternalInput").ap()
    den_d = nc.dram_tensor("den", [128 * LT], F32, kind="ExternalOutput").ap()
    num_d = nc.dram_tensor("num", [128 * LT], F32, kind="ExternalOutput").ap()
    stail_d = nc.dram_tensor("stail", [TW * L_PAD], F32, kind="ExternalOutput").ap()

    with tile.TileContext(nc) as tc:
        _kernel_body(tc, ids_d, emb_d, w_d, cb_d, ut_d, den_d, num_d, stail_d)
    nc.compile()
    return nc


def _kernel_body(tc, ids_d, emb_d, w_d, cb_d, ut_d, den_d, num_d, stail_d):
    nc = tc.nc
    from contextlib import ExitStack

    ctx = ExitStack()
    with ctx:
        persist = ctx.enter_context(tc.tile_pool(name="persist", bufs=1))
        epool = ctx.enter_context(tc.tile_pool(name="epool", bufs=6))

        # ---- persistent tiles / input DMAs ----
        ids_sb = persist.tile([128, NTB], I32, name="ids_sb", tag="ids_sb")
        nc.sync.dma_start(out=ids_sb[:], in_=ids_d.rearrange("(n p) -> p n", p=128))

        cb_sb = persist.tile([128, NDC], F32, name="cb_sb", tag="cb_sb")
        nc.sync.dma_start(out=cb_sb[:], in_=cb_d.rearrange("c p -> p c"))

        ident = persist.tile([128, 128], F32, name="ident", tag="ident")
        make_identity(nc, ident[:])

        w_sb = []
        for ic in range(NDC):
            wt = persist.tile([128, K * D_PAD], F16, name=f"w_sb{ic}", tag=f"w_sb{ic}")
            nc.sync.dma_start(out=wt[:], in_=w_d[ic])
            w_sb.append(wt)

        # all of U resident: [d_par 128, lt 70, dc 3, l 128]
        ut_all = persist.tile([128, LT, NDC, 128], F16, name="ut_all", tag="ut_all")
        nc.sync.dma_start(out=ut_all[:], in_=ut_d)

        x_sb = []
        for ic in range(NDC):
            xt = persist.tile([128, T_G], F16, name=f"x_sb{ic}", tag=f"x_sb{ic}")
            x_sb.append(xt)
        # partitions 44..127 of the last d-chunk are zero so conv matmuls can
        # contract the full 128 partitions (the gather copies then overwrite
        # rows 0..43)
        nc.vector.memset(x_sb[2][:, :], 0.0)

        h_sb = []
        for oc in range(NDC):
            ht = persist.tile([128, TP_PAD], F16, name=f"h_sb{oc}", tag=f"h_sb{oc}")
            h_sb.append(ht)

        den_sb = persist.tile([128, LT], F32, name="den_sb", tag="den_sb")
        num_sb = persist.tile([128, LT], F32, name="num_sb", tag="num_sb")
        stail_sb = persist.tile([TW, L_PAD], F32, name="stail_sb", tag="stail_sb")
        junk_sb = persist.tile([128, T], BF16, name="junk_sb", tag="junk_sb")

        # preload the exp activation table during the embed phase (~2.7us)
        warm = persist.tile([128, 1], F32, name="warm", tag="warm")
        nc.scalar.activation(out=warm[:], in_=ident[:, 0:1],
                             func=mybir.ActivationFunctionType.Exp)

        # ---- phase 1+2: embedding gather/transpose interleaved with conv ----
        def issue_gather(tb):
            e_t = epool.tile([128, D], F32, name=f"e_t{tb}", tag="e_t")
            nc.gpsimd.indirect_dma_start(
                out=e_t[:],
                out_offset=None,
                in_=emb_d,
                in_offset=bass.IndirectOffsetOnAxis(ap=ids_sb[:, tb:tb + 1], axis=0),
            )
            tp_ps = pp.tile([128, 512], F32, name=f"tp{tb}", tag="ps", space="PSUM")
            for dc in range(NDC):
                dp = DPAR[dc]
                nc.tensor.transpose(
                    out=tp_ps[:dp, dc * 128:dc * 128 + 128],
                    in_=e_t[:, dc * 128:dc * 128 + dp],
                    identity=ident[:],
                )
                nc.vector.tensor_copy(
                    out=x_sb[dc][:dp, tb * 128:(tb + 1) * 128],
                    in_=tp_ps[:dp, dc * 128:dc * 128 + 128],
                )

        def issue_conv_block(t0, tw):
            for oc in range(NDC):
                ps = pp.tile([128, 512], F32, name=f"cv{oc}_{t0}", tag="ps", space="PSUM")
                imm = 0
                for k in range(K):
                    for ic in range(NDC):
                        nc.tensor.matmul(
                            out=ps[:, :tw],
                            lhsT=w_sb[ic][:, k * D_PAD + oc * 128:k * D_PAD + (oc + 1) * 128],
                            rhs=x_sb[ic][:, t0 + k:t0 + k + tw],
                            start=(imm == 0),
                            stop=(imm == K * NDC - 1),
                        )
                        imm += 1
                nc.scalar.activation(
                    out=h_sb[oc][:, t0:t0 + tw],
                    in_=ps[:, :tw],
                    func=mybir.ActivationFunctionType.Relu,
                    bias=cb_sb[:, oc:oc + 1],
                    scale=1.0,
                )

        with tc.tile_pool(name="pp", bufs=4, space="PSUM") as pp:
            done_tb = 0
            for bi, (t0, tw) in enumerate(CONV_BLOCKS):
                need = CONV_NEED_TB[bi]
                while done_tb < need:
                    issue_gather(done_tb)
                    done_tb += 1
                issue_conv_block(t0, tw)

        # ---- phase 3: transposed tail scores  s_tail^T [9, L_PAD] ----
        # stationary = h tail slab [128(d), 9], moving = resident U^T, so the
        # weight loads are 9 columns (~free) and U streams once per d-chunk.
        with tc.tile_pool(name="tailp", bufs=2, space="PSUM") as tailp:
            CHUNK_LT = 16  # 16 l-tiles = 2048 cols per psum tile
            for lt0 in range(0, LT, CHUNK_LT):
                nlt = min(CHUNK_LT, LT - lt0)
                ps = tailp.tile([128, 2048], F32, name=f"tl{lt0}", tag="tl", space="PSUM")
                for sub in range(0, nlt, 4):
                    w = min(4, nlt - sub) * 128
                    for dc in range(NDC):
                        nc.tensor.matmul(
                            out=ps[:TW, sub * 128:sub * 128 + w],
                            lhsT=h_sb[dc][:, T:T + TW],
                            rhs=ut_all[:, lt0 + sub:lt0 + sub + w // 128, dc, :],
                            start=(dc == 0),
                            stop=(dc == NDC - 1),
                        )
                nc.vector.tensor_copy(
                    out=stail_sb[:, lt0 * 128:(lt0 + nlt) * 128],
                    in_=ps[:TW, :nlt * 128],
                )
            nc.sync.dma_start(
                out=stail_d.rearrange("(p n) -> p n", p=TW), in_=stail_sb[:]
            )

        # ---- phase 4: main scores, t = 0..2047, one l-tile at a time ----
        with tc.tile_pool(name="spool", bufs=2, space="PSUM") as spool, \
             tc.tile_pool(name="ppool", bufs=2) as ppool:
            for lt in range(LT):
                ps = spool.tile([128, T], F32, name=f"sc{lt}", tag="sc", space="PSUM")
                for dc in range(NDC):
                    for blk in range(4):
                        nc.tensor.matmul(
                            out=ps[:, blk * 512:(blk + 1) * 512],
                            lhsT=ut_all[:, lt, dc, :],
                            rhs=h_sb[dc][:, blk * 512:(blk + 1) * 512],
                            start=(dc == 0),
                            stop=(dc == NDC - 1),
                        )
                p_t = ppool.tile([128, T], BF16, name=f"p{lt}", tag="p_t")
                nc.scalar.activation(
                    out=p_t[:],
                    in_=ps[:],
                    func=mybir.ActivationFunctionType.Exp,
                    accum_out=den_sb[:, lt:lt + 1],
                )
                nc.vector.scalar_tensor_tensor(
                    out=junk_sb[:],
                    in0=ps[:],
                    scalar=1.0,
                    in1=p_t[:],
                    op0=mybir.AluOpType.mult,
                    op1=mybir.AluOpType.mult,
                    accum_out=num_sb[:, lt:lt + 1],
                )

        nc.sync.dma_start(out=den_d.rearrange("(p n) -> p n", n=LT), in_=den_sb[:])
        nc.sync.dma_start(out=num_d.rearrange("(p n) -> p n", n=LT), in_=num_sb[:])


def _prep_inputs(ids, embed_w, conv_w, conv_b, U, fc_bias):
    ids = np.ascontiguousarray(np.asarray(ids, dtype=np.int32))
    embed_w = np.ascontiguousarray(np.asarray(embed_w, dtype=np.float32))
    conv_w = np.asarray(conv_w, dtype=np.float32)
    conv_b = np.asarray(conv_b, dtype=np.float32)
    U = np.asarray(U, dtype=np.float32)

    # conv weights -> [ic, i_par, k, o_pad]; lhsT slice [i, o] per (k, oc)
    w_prep = np.zeros((NDC, 128, K, D_PAD), np.float32)
    cw = conv_w.transpose(1, 2, 0)  # (i, k, o)
    for ic in range(NDC):
        ip = DPAR[ic]
        w_prep[ic, :ip, :, :D] = cw[ic * 128:ic * 128 + ip]
    w_prep = np.ascontiguousarray(w_prep.reshape(NDC, 128, K * D_PAD).astype(np.float16))

    cb_prep = np.zeros((NDC, 128), np.float32)
    cb_prep.reshape(-1)[:D] = conv_b

    Upad = np.zeros((L_PAD, D_PAD), np.float32)
    Upad[:L, :D] = U
    # [p(d), lt, dc, l]
    ut_prep = np.ascontiguousarray(
        Upad.reshape(LT, 128, NDC, 128).transpose(3, 0, 2, 1)
        .reshape(128, LT * NDC * 128).astype(np.float16)
    )

    common = {
        "embed_w": embed_w,
        "w_prep": w_prep,
        "cb_prep": cb_prep,
        "ut_prep": ut_prep,
    }
    ids_pad = np.zeros((B, T_G), np.int32)
    ids_pad[:, K - 1:K - 1 + T] = ids
    return [dict(common, ids=np.ascontiguousarray(ids_pad[b])) for b in range(B)]


def _finalize(res_b, fc_bias):
    """Combine per-core partials into logits[L] (float64 on host)."""
    den_m = res_b["den"].astype(np.float64).reshape(128, LT)
    num_m = res_b["num"].astype(np.float64).reshape(128, LT)
    stail = res_b["stail"].astype(np.float64).reshape(TW, L_PAD)
    p_t = np.exp(stail)
    den = den_m.T.reshape(L_PAD) + p_t.sum(axis=0)
    num = num_m.T.reshape(L_PAD) + (stail * p_t).sum(axis=0)
    logits = num[:L] / den[:L] + np.asarray(fc_bias, np.float64)
    return logits.astype(np.float32)


def get_bass():
    if "nc" not in _BUILT:
        _BUILT["nc"] = _build_bass()
    return _BUILT["nc"]


def kernel(ids, embed_w, conv_w, conv_b, U, fc_bias):
    nc = get_bass()
    in_maps = _prep_inputs(ids, embed_w, conv_w, conv_b, U, fc_bias)
    res = run_bass_kernel_spmd(nc, in_maps, list(range(B))).results
    fcb = np.asarray(fc_bias, np.float32)
    return np.stack([_finalize(res[b], fcb) for b in range(B)], axis=0)


# revision 9
# speedup vs baseline: 1.1682x; 1.1682x over previous
"""Trainium2 Bass kernel for nn_DP_CAML_33646773797448 (sparse_attention).

Reference computation (per batch b):
    e      = embed_w[ids[b]]                       # (T, D)
    x      = e.T                                   # (D, T)
    h      = relu(conv1d(x, conv_w, pad=K-1) + b)  # (D, T')  T' = T + K - 1
    s      = U @ h                                 # (L, T')  raw scores
    attn   = softmax(s, axis=-1)
    z      = attn @ h.T                            # (L, D)
    logits = sum_d z * U + fc_bias                 # (L,)

Key identity: logits[l] = sum_t softmax(s)[l,t] * s[l,t], i.e. a
softmax-weighted mean of the raw scores. The kernel computes, per l-tile of
128 rows, den[l] = sum_t exp(s) (scalar-engine accumulate during the exp
pass) and num[l] = sum_t s*exp(s) (vector-engine scalar_tensor_tensor with
accumulate). The last 9 score columns (t=2048..2056) are computed by a
separate transposed matmul (stationary = h_tail so the weight loads are
9-column, i.e. free) and the raw 9 x L scores are shipped to the host, which
does the final (num+num_tail)/(den+den_tail) + bias combine while stacking
the 8 per-core outputs.

Sharding: pure data-parallel over B (B == 8 == n_cores), no collectives.
"""

import numpy as np

import concourse.bass as bass
import concourse.tile as tile
from concourse import bacc
from concourse import mybir
from concourse.bass_utils import run_bass_kernel_spmd
from concourse.masks import make_identity

F32 = mybir.dt.float32
F16 = mybir.dt.float16
BF16 = mybir.dt.bfloat16
I32 = mybir.dt.int32

# Problem shapes (hardcoded per contract)
VOCAB, L, D, K = 50000, 8921, 300, 10
B, T = 8, 2048
TP = T + K - 1            # 2057 conv output length
TP_PAD = TP + 1           # 2058 (the junk col 2057 is never read by stats)
NTB = 17                  # gather blocks of 128 tokens (ids padded with token 0)
T_G = NTB * 128           # 2176 gathered tokens
DPAR = [128, 128, 44]     # D = 300 split into partition chunks
NDC = 3
LT = (L + 127) // 128     # 70 l-tiles (last one has 89 valid rows)
L_PAD = LT * 128          # 8960
D_PAD = NDC * 128         # 384
TW = 9                    # tail columns t = 2048..2056

# conv t'-blocks (equal-ish, <= 512 so each fits a PSUM bank)
CONV_BLOCKS = [(0, 412), (412, 412), (824, 412), (1236, 412), (1648, 410)]
# gathers that must be complete before conv block i can run:
# block i needs x cols [t0, t0+tw+K-1) -> tokens up to t0+tw+8
CONV_NEED_TB = [4, 7, 10, 13, 17]

_BUILT = {}


def _build_bass():
    nc = bacc.Bacc("TRN2", target_bir_lowering=False, debug=False)

    ids_d = nc.dram_tensor("ids", [T_G], I32, kind="ExternalInput").ap()
    emb_d = nc.dram_tensor("embed_w", [VOCAB, D], F32, kind="ExternalInput").ap()
    w_d = nc.dram_tensor("w_prep", [NDC, 128, K * D_PAD], F16, kind="ExternalInput").ap()
    cb_d = nc.dram_tensor("cb_prep", [NDC, 128], F32, kind="ExternalInput").ap()
    ut_d = nc.dram_tensor("ut_prep", [128, LT * NDC * 128], F16, kind="ExternalInput").ap()
    den_d = nc.dram_tensor("den", [128 * 2 * LT], F32, kind="ExternalOutput").ap()
    num_d = nc.dram_tensor("num", [128 * 2 * LT], F32, kind="ExternalOutput").ap()
    stail_d = nc.dram_tensor("stail", [TW * L_PAD], F32, kind="ExternalOutput").ap()

    with tile.TileContext(nc) as tc:
        _kernel_body(tc, ids_d, emb_d, w_d, cb_d, ut_d, den_d, num_d, stail_d)
    nc.compile()
    return nc


def _kernel_body(tc, ids_d, emb_d, w_d, cb_d, ut_d, den_d, num_d, stail_d):
    nc = tc.nc
    from contextlib import ExitStack

    ctx = ExitStack()
    with ctx:
        persist = ctx.enter_context(tc.tile_pool(name="persist", bufs=1))
        epool = ctx.enter_context(tc.tile_pool(name="epool", bufs=6))

        # ---- persistent tiles / input DMAs ----
        ids_sb = persist.tile([128, NTB], I32, name="ids_sb", tag="ids_sb")
        nc.sync.dma_start(out=ids_sb[:], in_=ids_d.rearrange("(n p) -> p n", p=128))

        cb_sb = persist.tile([128, NDC], F32, name="cb_sb", tag="cb_sb")
        nc.sync.dma_start(out=cb_sb[:], in_=cb_d.rearrange("c p -> p c"))

        ident = persist.tile([128, 128], F32, name="ident", tag="ident")
        make_identity(nc, ident[:])

        w_sb = []
        for ic in range(NDC):
            wt = persist.tile([128, K * D_PAD], F16, name=f"w_sb{ic}", tag=f"w_sb{ic}")
            nc.sync.dma_start(out=wt[:], in_=w_d[ic])
            w_sb.append(wt)

        # all of U resident: [d_par 128, lt 70, dc 3, l 128]
        ut_all = persist.tile([128, LT, NDC, 128], F16, name="ut_all", tag="ut_all")
        nc.sync.dma_start(out=ut_all[:], in_=ut_d)

        x_sb = []
        for ic in range(NDC):
            xt = persist.tile([128, T_G], F16, name=f"x_sb{ic}", tag=f"x_sb{ic}")
            x_sb.append(xt)
        # partitions 44..127 of the last d-chunk are zero so conv matmuls can
        # contract the full 128 partitions (the gather copies then overwrite
        # rows 0..43)
        nc.vector.memset(x_sb[2][:, :], 0.0)

        h_sb = []
        for oc in range(NDC):
            ht = persist.tile([128, TP_PAD], F16, name=f"h_sb{oc}", tag=f"h_sb{oc}")
            h_sb.append(ht)

        den_sb = persist.tile([128, 2 * LT], F32, name="den_sb", tag="den_sb")
        num_sb = persist.tile([128, 2 * LT], F32, name="num_sb", tag="num_sb")
        stail_sb = persist.tile([TW, L_PAD], F32, name="stail_sb", tag="stail_sb")
        junk_sb = persist.tile([128, T], BF16, name="junk_sb", tag="junk_sb")

        # preload the exp activation table during the embed phase (~2.7us)
        warm = persist.tile([128, 1], F32, name="warm", tag="warm")
        nc.scalar.activation(out=warm[:], in_=ident[:, 0:1],
                             func=mybir.ActivationFunctionType.Exp)

        # ---- phase 1+2: embedding gather/transpose interleaved with conv ----
        def issue_gather(tb):
            e_t = epool.tile([128, D], F32, name=f"e_t{tb}", tag="e_t")
            nc.gpsimd.indirect_dma_start(
                out=e_t[:],
                out_offset=None,
                in_=emb_d,
                in_offset=bass.IndirectOffsetOnAxis(ap=ids_sb[:, tb:tb + 1], axis=0),
            )
            tp_ps = pp.tile([128, 512], F32, name=f"tp{tb}", tag="ps", space="PSUM")
            for dc in range(NDC):
                dp = DPAR[dc]
                nc.tensor.transpose(
                    out=tp_ps[:dp, dc * 128:dc * 128 + 128],
                    in_=e_t[:, dc * 128:dc * 128 + dp],
                    identity=ident[:],
                )
                nc.vector.tensor_copy(
                    out=x_sb[dc][:dp, tb * 128:(tb + 1) * 128],
                    in_=tp_ps[:dp, dc * 128:dc * 128 + 128],
                )

        def issue_conv_block(t0, tw):
            for oc in range(NDC):
                ps = pp.tile([128, 512], F32, name=f"cv{oc}_{t0}", tag="ps", space="PSUM")
                imm = 0
                for k in range(K):
                    for ic in range(NDC):
                        nc.tensor.matmul(
                            out=ps[:, :tw],
                            lhsT=w_sb[ic][:, k * D_PAD + oc * 128:k * D_PAD + (oc + 1) * 128],
                            rhs=x_sb[ic][:, t0 + k:t0 + k + tw],
                            start=(imm == 0),
                            stop=(imm == K * NDC - 1),
                        )
                        imm += 1
                nc.scalar.activation(
                    out=h_sb[oc][:, t0:t0 + tw],
                    in_=ps[:, :tw],
                    func=mybir.ActivationFunctionType.Relu,
                    bias=cb_sb[:, oc:oc + 1],
                    scale=1.0,
                )

        with tc.tile_pool(name="pp", bufs=4, space="PSUM") as pp:
            done_tb = 0
            for bi, (t0, tw) in enumerate(CONV_BLOCKS):
                need = CONV_NEED_TB[bi]
                while done_tb < need:
                    issue_gather(done_tb)
                    done_tb += 1
                issue_conv_block(t0, tw)

        # ---- phase 3: transposed tail scores  s_tail^T [9, L_PAD] ----
        # stationary = h tail slab [128(d), 9], moving = resident U^T, so the
        # weight loads are 9 columns (~free) and U streams once per d-chunk.
        with tc.tile_pool(name="tailp", bufs=2, space="PSUM") as tailp:
            CHUNK_LT = 16  # 16 l-tiles = 2048 cols per psum tile
            for lt0 in range(0, LT, CHUNK_LT):
                nlt = min(CHUNK_LT, LT - lt0)
                ps = tailp.tile([128, 2048], F32, name=f"tl{lt0}", tag="tl", space="PSUM")
                for sub in range(0, nlt, 4):
                    w = min(4, nlt - sub) * 128
                    for dc in range(NDC):
                        nc.tensor.matmul(
                            out=ps[:TW, sub * 128:sub * 128 + w],
                            lhsT=h_sb[dc][:, T:T + TW],
                            rhs=ut_all[:, lt0 + sub:lt0 + sub + w // 128, dc, :],
                            start=(dc == 0),
                            stop=(dc == NDC - 1),
                        )
                nc.vector.tensor_copy(
                    out=stail_sb[:, lt0 * 128:(lt0 + nlt) * 128],
                    in_=ps[:TW, :nlt * 128],
                )
            nc.sync.dma_start(
                out=stail_d.rearrange("(p n) -> p n", p=TW), in_=stail_sb[:]
            )

        # ---- phase 4: main scores, t = 0..2047, two 1024-chunks per l-tile ----
        # 1024-wide psum tiles x 4 slots give pipeline depth 4 so the serial
        # MM -> exp -> stt chain per tile overlaps across tiles.
        with tc.tile_pool(name="spool", bufs=4, space="PSUM") as spool, \
             tc.tile_pool(name="ppool", bufs=4) as ppool:
            for lt in range(LT):
                pss = [
                    spool.tile([128, 1024], F32, name=f"sc{lt}_{ck}", tag="sc",
                               space="PSUM")
                    for ck in range(2)
                ]
                for dc in range(NDC):
                    for ck in range(2):
                        for blk in range(2):
                            nc.tensor.matmul(
                                out=pss[ck][:, blk * 512:(blk + 1) * 512],
                                lhsT=ut_all[:, lt, dc, :],
                                rhs=h_sb[dc][:, ck * 1024 + blk * 512:
                                             ck * 1024 + (blk + 1) * 512],
                                start=(dc == 0),
                                stop=(dc == NDC - 1),
                            )
                for ck in range(2):
                    col = 2 * lt + ck
                    p_t = ppool.tile([128, 1024], BF16, name=f"p{lt}_{ck}", tag="p_t")
                    nc.scalar.activation(
                        out=p_t[:],
                        in_=pss[ck][:],
                        func=mybir.ActivationFunctionType.Exp,
                        accum_out=den_sb[:, col:col + 1],
                    )
                    nc.vector.scalar_tensor_tensor(
                        out=junk_sb[:, :1024],
                        in0=pss[ck][:],
                        scalar=1.0,
                        in1=p_t[:],
                        op0=mybir.AluOpType.mult,
                        op1=mybir.AluOpType.mult,
                        accum_out=num_sb[:, col:col + 1],
                    )

        nc.sync.dma_start(out=den_d.rearrange("(p n) -> p n", n=2 * LT), in_=den_sb[:])
        nc.sync.dma_start(out=num_d.rearrange("(p n) -> p n", n=2 * LT), in_=num_sb[:])


def _prep_inputs(ids, embed_w, conv_w, conv_b, U, fc_bias):
    ids = np.ascontiguousarray(np.asarray(ids, dtype=np.int32))
    embed_w = np.ascontiguousarray(np.asarray(embed_w, dtype=np.float32))
    conv_w = np.asarray(conv_w, dtype=np.float32)
    conv_b = np.asarray(conv_b, dtype=np.float32)
    U = np.asarray(U, dtype=np.float32)

    # conv weights -> [ic, i_par, k, o_pad]; lhsT slice [i, o] per (k, oc)
    w_prep = np.zeros((NDC, 128, K, D_PAD), np.float32)
    cw = conv_w.transpose(1, 2, 0)  # (i, k, o)
    for ic in range(NDC):
        ip = DPAR[ic]
        w_prep[ic, :ip, :, :D] = cw[ic * 128:ic * 128 + ip]
    w_prep = np.ascontiguousarray(w_prep.reshape(NDC, 128, K * D_PAD).astype(np.float16))

    cb_prep = np.zeros((NDC, 128), np.float32)
    cb_prep.reshape(-1)[:D] = conv_b

    Upad = np.zeros((L_PAD, D_PAD), np.float32)
    Upad[:L, :D] = U
    # [p(d), lt, dc, l]
    ut_prep = np.ascontiguousarray(
        Upad.reshape(LT, 128, NDC, 128).transpose(3, 0, 2, 1)
        .reshape(128, LT * NDC * 128).astype(np.float16)
    )

    common = {
        "embed_w": embed_w,
        "w_prep": w_prep,
        "cb_prep": cb_prep,
        "ut_prep": ut_prep,
    }
    ids_pad = np.zeros((B, T_G), np.int32)
    ids_pad[:, K - 1:K - 1 + T] = ids
    return [dict(common, ids=np.ascontiguousarray(ids_pad[b])) for b in range(B)]


def _finalize(res_b, fc_bias):
    """Combine per-core partials into logits[L] (float64 on host)."""
    den_m = res_b["den"].astype(np.float64).reshape(128, LT, 2).sum(axis=2)
    num_m = res_b["num"].astype(np.float64).reshape(128, LT, 2).sum(axis=2)
    stail = res_b["stail"].astype(np.float64).reshape(TW, L_PAD)
    p_t = np.exp(stail)
    den = den_m.T.reshape(L_PAD) + p_t.sum(axis=0)
    num = num_m.T.reshape(L_PAD) + (stail * p_t).sum(axis=0)
    logits = num[:L] / den[:L] + np.asarray(fc_bias, np.float64)
    return logits.astype(np.float32)


def get_bass():
    if "nc" not in _BUILT:
        _BUILT["nc"] = _build_bass()
    return _BUILT["nc"]


def kernel(ids, embed_w, conv_w, conv_b, U, fc_bias):
    nc = get_bass()
    in_maps = _prep_inputs(ids, embed_w, conv_w, conv_b, U, fc_bias)
    res = run_bass_kernel_spmd(nc, in_maps, list(range(B))).results
    fcb = np.asarray(fc_bias, np.float32)
    return np.stack([_finalize(res[b], fcb) for b in range(B)], axis=0)


# revision 12
# speedup vs baseline: 1.2007x; 1.0278x over previous
"""Trainium2 Bass kernel for nn_DP_CAML_33646773797448 (sparse_attention).

Reference computation (per batch b):
    e      = embed_w[ids[b]]                       # (T, D)
    x      = e.T                                   # (D, T)
    h      = relu(conv1d(x, conv_w, pad=K-1) + b)  # (D, T')  T' = T + K - 1
    s      = U @ h                                 # (L, T')  raw scores
    attn   = softmax(s, axis=-1)
    z      = attn @ h.T                            # (L, D)
    logits = sum_d z * U + fc_bias                 # (L,)

Key identity: logits[l] = sum_t softmax(s)[l,t] * s[l,t], i.e. a
softmax-weighted mean of the raw scores. The kernel computes, per l-tile of
128 rows, den[l] = sum_t exp(s) (scalar-engine accumulate during the exp
pass) and num[l] = sum_t s*exp(s) (vector-engine scalar_tensor_tensor with
accumulate). The last 9 score columns (t=2048..2056) are computed by a
separate transposed matmul (stationary = h_tail so the weight loads are
9-column, i.e. free) and the raw 9 x L scores are shipped to the host, which
does the final (num+num_tail)/(den+den_tail) + bias combine while stacking
the 8 per-core outputs.

Sharding: pure data-parallel over B (B == 8 == n_cores), no collectives.
"""

import numpy as np

import concourse.bass as bass
import concourse.tile as tile
from concourse import bacc
from concourse import mybir
from concourse.bass_utils import run_bass_kernel_spmd
from concourse.masks import make_identity

F32 = mybir.dt.float32
F16 = mybir.dt.float16
BF16 = mybir.dt.bfloat16
I32 = mybir.dt.int32

# Problem shapes (hardcoded per contract)
VOCAB, L, D, K = 50000, 8921, 300, 10
B, T = 8, 2048
TP = T + K - 1            # 2057 conv output length
TP_PAD = TP + 1           # 2058 (the junk col 2057 is never read by stats)
NTB = 17                  # gather blocks of 128 tokens (ids padded with token 0)
T_G = NTB * 128           # 2176 gathered tokens
DPAR = [128, 128, 44]     # D = 300 split into partition chunks
NDC = 3
LT = (L + 127) // 128     # 70 l-tiles (last one has 89 valid rows)
L_PAD = LT * 128          # 8960
D_PAD = NDC * 128         # 384
TW = 9                    # tail columns t = 2048..2056

# conv t'-blocks (equal-ish, <= 512 so each fits a PSUM bank)
CONV_BLOCKS = [(0, 412), (412, 412), (824, 412), (1236, 412), (1648, 410)]
# gathers that must be complete before conv block i can run:
# block i needs x cols [t0, t0+tw+K-1) -> tokens up to t0+tw+8
CONV_NEED_TB = [4, 7, 10, 13, 17]

_BUILT = {}


def _build_bass():
    nc = bacc.Bacc("TRN2", target_bir_lowering=False, debug=False)

    ids_d = nc.dram_tensor("ids", [T_G], I32, kind="ExternalInput").ap()
    emb_d = nc.dram_tensor("embed_w", [VOCAB, D], F32, kind="ExternalInput").ap()
    w_d = nc.dram_tensor("w_prep", [NDC, 128, K * D_PAD], F16, kind="ExternalInput").ap()
    cb_d = nc.dram_tensor("cb_prep", [NDC, 128], F32, kind="ExternalInput").ap()
    ut_d = nc.dram_tensor("ut_prep", [128, LT * NDC * 128], F16, kind="ExternalInput").ap()
    den_d = nc.dram_tensor("den", [128 * 2 * LT], F32, kind="ExternalOutput").ap()
    num_d = nc.dram_tensor("num", [128 * 2 * LT], F32, kind="ExternalOutput").ap()
    stail_d = nc.dram_tensor("stail", [TW * L_PAD], F32, kind="ExternalOutput").ap()

    with tile.TileContext(nc) as tc:
        _kernel_body(tc, ids_d, emb_d, w_d, cb_d, ut_d, den_d, num_d, stail_d)
    nc.compile()
    return nc


def _kernel_body(tc, ids_d, emb_d, w_d, cb_d, ut_d, den_d, num_d, stail_d):
    nc = tc.nc
    from contextlib import ExitStack

    ctx = ExitStack()
    with ctx:
        persist = ctx.enter_context(tc.tile_pool(name="persist", bufs=1))
        epool = ctx.enter_context(tc.tile_pool(name="epool", bufs=NTB))

        # ---- persistent tiles / input DMAs ----
        ids_sb = persist.tile([128, NTB], I32, name="ids_sb", tag="ids_sb")
        nc.sync.dma_start(out=ids_sb[:], in_=ids_d.rearrange("(n p) -> p n", p=128))

        cb_sb = persist.tile([128, NDC], F32, name="cb_sb", tag="cb_sb")
        nc.sync.dma_start(out=cb_sb[:], in_=cb_d.rearrange("c p -> p c"))

        ident = persist.tile([128, 128], F32, name="ident", tag="ident")
        make_identity(nc, ident[:])

        w_sb = []
        for ic in range(NDC):
            wt = persist.tile([128, K * D_PAD], F16, name=f"w_sb{ic}", tag=f"w_sb{ic}")
            nc.sync.dma_start(out=wt[:], in_=w_d[ic])
            w_sb.append(wt)

        # all of U resident: [d_par 128, lt 70, dc 3, l 128]
        ut_all = persist.tile([128, LT, NDC, 128], F16, name="ut_all", tag="ut_all")
        nc.sync.dma_start(out=ut_all[:], in_=ut_d)

        x_sb = []
        for ic in range(NDC):
            xt = persist.tile([128, T_G], F16, name=f"x_sb{ic}", tag=f"x_sb{ic}")
            x_sb.append(xt)
        # partitions 44..127 of the last d-chunk are zero so conv matmuls can
        # contract the full 128 partitions (the gather copies then overwrite
        # rows 0..43)
        nc.vector.memset(x_sb[2][:, :], 0.0)

        h_sb = []
        for oc in range(NDC):
            ht = persist.tile([128, TP_PAD], F16, name=f"h_sb{oc}", tag=f"h_sb{oc}")
            h_sb.append(ht)

        den_sb = persist.tile([128, 2 * LT], F32, name="den_sb", tag="den_sb")
        num_sb = persist.tile([128, 2 * LT], F32, name="num_sb", tag="num_sb")
        stail_sb = persist.tile([TW, L_PAD], F32, name="stail_sb", tag="stail_sb")
        junk_sb = persist.tile([128, T], BF16, name="junk_sb", tag="junk_sb")

        # preload the exp activation table during the embed phase (~2.7us)
        warm = persist.tile([128, 1], F32, name="warm", tag="warm")
        nc.scalar.activation(out=warm[:], in_=ident[:, 0:1],
                             func=mybir.ActivationFunctionType.Exp)

        # ---- phase 1+2: embedding gather/transpose interleaved with conv ----
        # issue every gather DMA up-front so the (serial) gpsimd DMA queue
        # starts immediately; transposes/conv consume them as they land
        e_ts = []
        for tb in range(NTB):
            e_t = epool.tile([128, D], F32, name=f"e_t{tb}", tag="e_t")
            nc.gpsimd.indirect_dma_start(
                out=e_t[:],
                out_offset=None,
                in_=emb_d,
                in_offset=bass.IndirectOffsetOnAxis(ap=ids_sb[:, tb:tb + 1], axis=0),
            )
            e_ts.append(e_t)

        def issue_transpose(tb):
            e_t = e_ts[tb]
            tp_ps = pp.tile([128, 512], F32, name=f"tp{tb}", tag="ps", space="PSUM")
            for dc in range(NDC):
                dp = DPAR[dc]
                nc.tensor.transpose(
                    out=tp_ps[:dp, dc * 128:dc * 128 + 128],
                    in_=e_t[:, dc * 128:dc * 128 + dp],
                    identity=ident[:],
                )
                nc.vector.tensor_copy(
                    out=x_sb[dc][:dp, tb * 128:(tb + 1) * 128],
                    in_=tp_ps[:dp, dc * 128:dc * 128 + 128],
                )

        def issue_conv_block(t0, tw):
            for oc in range(NDC):
                ps = pp.tile([128, 512], F32, name=f"cv{oc}_{t0}", tag="ps", space="PSUM")
                imm = 0
                for k in range(K):
                    for ic in range(NDC):
                        nc.tensor.matmul(
                            out=ps[:, :tw],
                            lhsT=w_sb[ic][:, k * D_PAD + oc * 128:k * D_PAD + (oc + 1) * 128],
                            rhs=x_sb[ic][:, t0 + k:t0 + k + tw],
                            start=(imm == 0),
                            stop=(imm == K * NDC - 1),
                        )
                        imm += 1
                nc.scalar.activation(
                    out=h_sb[oc][:, t0:t0 + tw],
                    in_=ps[:, :tw],
                    func=mybir.ActivationFunctionType.Relu,
                    bias=cb_sb[:, oc:oc + 1],
                    scale=1.0,
                )

        with tc.tile_pool(name="pp", bufs=4, space="PSUM") as pp:
            # pre-warm the PE so the HAM clock gate reaches 2.4 GHz before the
            # first conv matmul (junk matmuls on a zeroed tile)
            wz = persist.tile([128, 512], F16, name="wz", tag="wz")
            nc.vector.memset(wz[:, :], 0.0)
            wps = pp.tile([128, 512], F32, name="wps", tag="ps", space="PSUM")
            for i in range(14):
                nc.tensor.matmul(out=wps[:, :], lhsT=wz[:, :128], rhs=wz[:, :],
                                 start=(i == 0), stop=(i == 13))
            done_tb = 0
            for bi, (t0, tw) in enumerate(CONV_BLOCKS):
                need = CONV_NEED_TB[bi]
                while done_tb < need:
                    issue_transpose(done_tb)
                    done_tb += 1
                issue_conv_block(t0, tw)

        # ---- phase 3: transposed tail scores  s_tail^T [9, L_PAD] ----
        # stationary = h tail slab [128(d), 9], moving = resident U^T, so the
        # weight loads are 9 columns (~free) and U streams once per d-chunk.
        with tc.tile_pool(name="tailp", bufs=2, space="PSUM") as tailp:
            CHUNK_LT = 16  # 16 l-tiles = 2048 cols per psum tile
            for lt0 in range(0, LT, CHUNK_LT):
                nlt = min(CHUNK_LT, LT - lt0)
                ps = tailp.tile([128, 2048], F32, name=f"tl{lt0}", tag="tl", space="PSUM")
                for sub in range(0, nlt, 4):
                    w = min(4, nlt - sub) * 128
                    for dc in range(NDC):
                        nc.tensor.matmul(
                            out=ps[:TW, sub * 128:sub * 128 + w],
                            lhsT=h_sb[dc][:, T:T + TW],
                            rhs=ut_all[:, lt0 + sub:lt0 + sub + w // 128, dc, :],
                            start=(dc == 0),
                            stop=(dc == NDC - 1),
                        )
                nc.vector.tensor_copy(
                    out=stail_sb[:, lt0 * 128:(lt0 + nlt) * 128],
                    in_=ps[:TW, :nlt * 128],
                )
            nc.sync.dma_start(
                out=stail_d.rearrange("(p n) -> p n", p=TW), in_=stail_sb[:]
            )

        # ---- phase 4: main scores, t = 0..2047, two 1024-chunks per l-tile ----
        # 1024-wide psum tiles x 4 slots give pipeline depth 4 so the serial
        # MM -> exp -> stt chain per tile overlaps across tiles.
        with tc.tile_pool(name="spool", bufs=4, space="PSUM") as spool, \
             tc.tile_pool(name="ppool", bufs=4) as ppool:
            for lt in range(LT):
                pss = [
                    spool.tile([128, 1024], F32, name=f"sc{lt}_{ck}", tag="sc",
                               space="PSUM")
                    for ck in range(2)
                ]
                for dc in range(NDC):
                    for ck in range(2):
                        for blk in range(2):
                            nc.tensor.matmul(
                                out=pss[ck][:, blk * 512:(blk + 1) * 512],
                                lhsT=ut_all[:, lt, dc, :],
                                rhs=h_sb[dc][:, ck * 1024 + blk * 512:
                                             ck * 1024 + (blk + 1) * 512],
                                start=(dc == 0),
                                stop=(dc == NDC - 1),
                            )
                for ck in range(2):
                    col = 2 * lt + ck
                    p_t = ppool.tile([128, 1024], BF16, name=f"p{lt}_{ck}", tag="p_t")
                    nc.scalar.activation(
                        out=p_t[:],
                        in_=pss[ck][:],
                        func=mybir.ActivationFunctionType.Exp,
                        accum_out=den_sb[:, col:col + 1],
                    )
                    nc.vector.scalar_tensor_tensor(
                        out=junk_sb[:, :1024],
                        in0=pss[ck][:],
                        scalar=1.0,
                        in1=p_t[:],
                        op0=mybir.AluOpType.mult,
                        op1=mybir.AluOpType.mult,
                        accum_out=num_sb[:, col:col + 1],
                    )

        nc.sync.dma_start(out=den_d.rearrange("(p n) -> p n", n=2 * LT), in_=den_sb[:])
        nc.sync.dma_start(out=num_d.rearrange("(p n) -> p n", n=2 * LT), in_=num_sb[:])


def _prep_inputs(ids, embed_w, conv_w, conv_b, U, fc_bias):
    ids = np.ascontiguousarray(np.asarray(ids, dtype=np.int32))
    embed_w = np.ascontiguousarray(np.asarray(embed_w, dtype=np.float32))
    conv_w = np.asarray(conv_w, dtype=np.float32)
    conv_b = np.asarray(conv_b, dtype=np.float32)
    U = np.asarray(U, dtype=np.float32)

    # conv weights -> [ic, i_par, k, o_pad]; lhsT slice [i, o] per (k, oc)
    w_prep = np.zeros((NDC, 128, K, D_PAD), np.float32)
    cw = conv_w.transpose(1, 2, 0)  # (i, k, o)
    for ic in range(NDC):
        ip = DPAR[ic]
        w_prep[ic, :ip, :, :D] = cw[ic * 128:ic * 128 + ip]
    w_prep = np.ascontiguousarray(w_prep.reshape(NDC, 128, K * D_PAD).astype(np.float16))

    cb_prep = np.zeros((NDC, 128), np.float32)
    cb_prep.reshape(-1)[:D] = conv_b

    Upad = np.zeros((L_PAD, D_PAD), np.float32)
    Upad[:L, :D] = U
    # [p(d), lt, dc, l]
    ut_prep = np.ascontiguousarray(
        Upad.reshape(LT, 128, NDC, 128).transpose(3, 0, 2, 1)
        .reshape(128, LT * NDC * 128).astype(np.float16)
    )

    common = {
        "embed_w": embed_w,
        "w_prep": w_prep,
        "cb_prep": cb_prep,
        "ut_prep": ut_prep,
    }
    ids_pad = np.zeros((B, T_G), np.int32)
    ids_pad[:, K - 1:K - 1 + T] = ids
    return [dict(common, ids=np.ascontiguousarray(ids_pad[b])) for b in range(B)]


def _finalize(res_b, fc_bias):
    """Combine per-core partials into logits[L] (float64 on host)."""
    den_m = res_b["den"].astype(np.float64).reshape(128, LT, 2).sum(axis=2)
    num_m = res_b["num"].astype(np.float64).reshape(128, LT, 2).sum(axis=2)
    stail = res_b["stail"].astype(np.float64).reshape(TW, L_PAD)
    p_t = np.exp(stail)
    den = den_m.T.reshape(L_PAD) + p_t.sum(axis=0)
    num = num_m.T.reshape(L_PAD) + (stail * p_t).sum(axis=0)
    logits = num[:L] / den[:L] + np.asarray(fc_bias, np.float64)
    return logits.astype(np.float32)


def get_bass():
    if "nc" not in _BUILT:
        _BUILT["nc"] = _build_bass()
    return _BUILT["nc"]


def kernel(ids, embed_w, conv_w, conv_b, U, fc_bias):
    nc = get_bass()
    in_maps = _prep_inputs(ids, embed_w, conv_w, conv_b, U, fc_bias)
    res = run_bass_kernel_spmd(nc, in_maps, list(range(B))).results
    fcb = np.asarray(fc_bias, np.float32)
    return np.stack([_finalize(res[b], fcb) for b in range(B)], axis=0)
